# revision 4
# baseline (speedup 1.0000x reference)
"""CrossModalAttention Trainium2 kernel.

Full inputs in, full outputs out; internally sharded data-parallel over the
batch dim across 8 NeuronCores (4 batch items per core).

Per batch item (C=256, H=W=64, AS=8, T=64):
  R = avgpool8(F_rgb), D = avgpool8(F_d)           (DVE reduce_sum x2, 1/64
                                                    folded into the weights)
  Q = Wq@R+bq, K = Wk@D+bk  as [o, s]              (PE + ACT bias)
  VfT = D^T @ Wv^T + ones^T@bv  as [s, o]          (PE)
  A = Qf^T Kf  [t, s]; softmax rows                (PE + DVE + ACT exp)
  AsmT = softmax(A)^T                              (PE transpose)
  FattT = AsmT @ VfT  [t, c]                       (PE)
  out = alpha*up8(Fatt) + (1-alpha)*F_rgb: one accumulating PSUM group of
  two float32r matmuls per 512-wide block:
     psum  = FattT_chunk^T @ (alpha*kron(U^T,U^T))_block   (upsample)
     psum += ((1-alpha)*I)^T @ F_rgb_block                 (blend)
  then ACT copies psum back in-place into the F_rgb SBUF tile -> DMA out.
"""

import numpy as np
from contextlib import ExitStack

import concourse.bacc as bacc
import concourse.mybir as mybir
import concourse.tile as tile
from concourse.bass_utils import run_bass_kernel_spmd

B, C, H, W = 32, 256, 64, 64
AS = 8
T = AS * AS          # 64 pooled pixels
HW = H * W           # 4096
NCORES = 8
BPC = B // NCORES    # 4 batch items per core
NCHUNK = C // 128    # 2 channel chunks

F32 = mybir.dt.float32
F32R = mybir.dt.float32r


def _bilinear_up_matrix(n_out: int, n_in: int) -> np.ndarray:
    """U[i, p]: weight of coarse pixel p for fine pixel i; half-pixel centers
    with edge clamping (identical to jax.image.resize bilinear upsample)."""
    U = np.zeros((n_out, n_in), np.float64)
    scale = n_in / n_out
    for i in range(n_out):
        src = (i + 0.5) * scale - 0.5
        p0 = int(np.floor(src))
        f = src - p0
        for p, wgt in ((p0, 1.0 - f), (p0 + 1, f)):
            pc = min(max(p, 0), n_in - 1)
            U[i, pc] += wgt
    return U


_CACHE = {}


def _build_program():
    nc = bacc.Bacc("TRN2", target_bir_lowering=False, debug=False,
                   num_devices=NCORES)

    frgb = nc.dram_tensor("frgb", [BPC, NCHUNK, 128, HW], F32R,
                          kind="ExternalInput").ap()
    fd = nc.dram_tensor("fd", [BPC, NCHUNK, 128, HW], F32,
                        kind="ExternalInput").ap()
    wqt = nc.dram_tensor("wqt", [NCHUNK, 128, C], F32, kind="ExternalInput").ap()
    wkt = nc.dram_tensor("wkt", [NCHUNK, 128, C], F32, kind="ExternalInput").ap()
    wvt = nc.dram_tensor("wvt", [NCHUNK, 128, C], F32, kind="ExternalInput").ap()
    bq2 = nc.dram_tensor("bq2", [128, NCHUNK], F32, kind="ExternalInput").ap()
    bk2 = nc.dram_tensor("bk2", [128, NCHUNK], F32, kind="ExternalInput").ap()
    bvr = nc.dram_tensor("bvr", [1, C], F32, kind="ExternalInput").ap()
    u2a = nc.dram_tensor("u2a", [T, HW], F32R, kind="ExternalInput").ap()
    idsc = nc.dram_tensor("idsc", [128, 128], F32R, kind="ExternalInput").ap()
    id64 = nc.dram_tensor("id64", [T, T], F32, kind="ExternalInput").ap()
    ones64 = nc.dram_tensor("ones64", [1, T], F32, kind="ExternalInput").ap()
    out = nc.dram_tensor("out", [BPC, NCHUNK, 128, HW], F32R,
                         kind="ExternalOutput").ap()

    with tile.TileContext(nc) as tc, ExitStack() as ctx:
        consts = ctx.enter_context(tc.tile_pool(name="consts", bufs=1))
        fr_pool = ctx.enter_context(tc.tile_pool(name="fr", bufs=2))
        fd_pool = ctx.enter_context(tc.tile_pool(name="fdp", bufs=2))
        scr_pool = ctx.enter_context(tc.tile_pool(name="scr", bufs=4))
        small = ctx.enter_context(tc.tile_pool(name="small", bufs=2))
        ps_small = ctx.enter_context(
            tc.tile_pool(name="pss", bufs=4, space="PSUM"))
        ps_out = ctx.enter_context(
            tc.tile_pool(name="pso", bufs=3, space="PSUM"))

        # ---- constants into SBUF ----
        wqt_s = consts.tile([128, NCHUNK * C], F32)   # [c, (ci, o)]
        nc.sync.dma_start(wqt_s.rearrange("p (a b) -> p a b", a=NCHUNK),
                          wqt.transpose([1, 0, 2]))
        wkt_s = consts.tile([128, NCHUNK * C], F32)
        nc.sync.dma_start(wkt_s.rearrange("p (a b) -> p a b", a=NCHUNK),
                          wkt.transpose([1, 0, 2]))
        wvt_s = consts.tile([128, NCHUNK * C], F32)
        nc.sync.dma_start(wvt_s.rearrange("p (a b) -> p a b", a=NCHUNK),
                          wvt.transpose([1, 0, 2]))
        bq_s = consts.tile([128, NCHUNK], F32)
        nc.sync.dma_start(bq_s[:], bq2[:])
        bk_s = consts.tile([128, NCHUNK], F32)
        nc.sync.dma_start(bk_s[:], bk2[:])
        bvr_s = consts.tile([1, C], F32)
        nc.sync.dma_start(bvr_s[:], bvr[:])
        u2a_s = consts.tile([T, HW], F32R)
        nc.sync.dma_start(u2a_s[:], u2a[:])
        idsc_s = consts.tile([128, 128], F32R)
        nc.sync.dma_start(idsc_s[:], idsc[:])
        id64_s = consts.tile([T, T], F32)
        nc.sync.dma_start(id64_s[:], id64[:])
        ones_s = consts.tile([1, T], F32)
        nc.sync.dma_start(ones_s[:], ones64[:])

        for b in range(BPC):
            # ---- load F_rgb / F_d for this batch item ----
            fr_t = fr_pool.tile([128, NCHUNK * HW], F32R, tag="fr")
            nc.sync.dma_start(fr_t.rearrange("p (a b) -> p a b", a=NCHUNK),
                              frgb[b].transpose([1, 0, 2]))
            fd_t = fd_pool.tile([128, NCHUNK * HW], F32, tag="fd")
            nc.sync.dma_start(fd_t.rearrange("p (a b) -> p a b", a=NCHUNK),
                              fd[b].transpose([1, 0, 2]))

            # ---- avgpool (sums; /64 folded into weights) ----
            rs_t = small.tile([128, NCHUNK * T], F32, tag="rs")  # R sums
            ds_t = small.tile([128, NCHUNK * T], F32, tag="ds")  # D sums
            for src_t, dst_t in ((fr_t.bitcast(F32), rs_t), (fd_t, ds_t)):
                for ci in range(NCHUNK):
                    x = src_t[:, ci * HW:(ci + 1) * HW]
                    t3 = scr_pool.tile([128, 512], F32, tag="t3")
                    # sum over w within each 8-block (innermost)
                    nc.vector.reduce_sum(
                        t3[:], x.rearrange("p (s u) -> p s u", u=AS),
                        axis=mybir.AxisListType.X)
                    # t3 free index = h*8 + wq, h = hp*8 + v
                    # view as [hp, v, wq] then reorder to [hp, wq, v]
                    v_inner = t3.rearrange(
                        "p (a v w) -> p a v w", a=AS, v=AS).transpose([0, 1, 3, 2])
                    nc.vector.reduce_sum(
                        dst_t[:, ci * T:(ci + 1) * T], v_inner,
                        axis=mybir.AxisListType.X)

            # ---- Q, K: [o, s] with per-partition bias ----
            qf_t = small.tile([128, NCHUNK * T], F32, tag="qf")
            kf_t = small.tile([128, NCHUNK * T], F32, tag="kf")
            for w_s, b_s, sums, dst in ((wqt_s, bq_s, rs_t, qf_t),
                                        (wkt_s, bk_s, ds_t, kf_t)):
                for oj in range(NCHUNK):
                    psq = ps_small.tile([128, T], F32, tag="pss")
                    for ci in range(NCHUNK):
                        nc.tensor.matmul(
                            psq[:],
                            w_s[:, ci * C + oj * 128: ci * C + (oj + 1) * 128],
                            sums[:, ci * T:(ci + 1) * T],
                            start=(ci == 0), stop=(ci == NCHUNK - 1))
                    nc.scalar.activation(
                        dst[:, oj * T:(oj + 1) * T], psq[:],
                        mybir.ActivationFunctionType.Identity,
                        bias=b_s[:, oj:oj + 1], scale=1.0)

            # ---- VfT = D^T Wv^T + ones^T bv : [s, o] ----
            psv = ps_small.tile([T, C], F32, tag="pss")
            for ci in range(NCHUNK):
                nc.tensor.matmul(psv[:],
                                 ds_t[:, ci * T:(ci + 1) * T],
                                 wvt_s[:, ci * C:(ci + 1) * C],
                                 start=(ci == 0), stop=False)
            nc.tensor.matmul(psv[:], ones_s[:], bvr_s[:], start=False, stop=True)
            vft = small.tile([T, C], F32, tag="vft")
            nc.scalar.copy(vft[:], psv[:])

            # ---- A = Qf^T Kf : [t, s] ----
            psa = ps_small.tile([T, T], F32, tag="pss")
            for oj in range(NCHUNK):
                nc.tensor.matmul(psa[:],
                                 qf_t[:, oj * T:(oj + 1) * T],
                                 kf_t[:, oj * T:(oj + 1) * T],
                                 start=(oj == 0), stop=(oj == NCHUNK - 1))

            # ---- softmax over free dim ----
            negmax = small.tile([T, 1], F32, tag="negmax")
            nc.vector.tensor_reduce(negmax[:], psa[:],
                                    axis=mybir.AxisListType.X,
                                    op=mybir.AluOpType.max, negate=True)
            e_t = small.tile([T, T], F32, tag="e")
            nc.scalar.activation(e_t[:], psa[:],
                                 mybir.ActivationFunctionType.Exp,
                                 bias=negmax[:, 0:1], scale=1.0)
            s1 = small.tile([T, 1], F32, tag="s1")
            nc.vector.reduce_sum(s1[:], e_t[:], axis=mybir.AxisListType.X)
            r1 = small.tile([T, 1], F32, tag="r1")
            nc.vector.reciprocal(r1[:], s1[:])
            asm = small.tile([T, T], F32, tag="asm")
            nc.vector.tensor_scalar_mul(asm[:], e_t[:], r1[:, 0:1])

            # ---- AsmT via PE transpose ----
            psat = ps_small.tile([T, T], F32, tag="pss")
            nc.tensor.transpose(psat[:], asm[:], id64_s[:])
            asmt = small.tile([T, T], F32, tag="asmt")
            nc.scalar.copy(asmt[:], psat[:])

            # ---- FattT = AsmT @ VfT : [t, c] ----
            psf = ps_small.tile([T, C], F32, tag="pss")
            nc.tensor.matmul(psf[:], asmt[:], vft[:], start=True, stop=True)
            ft = small.tile([T, C], F32R, tag="ft")
            nc.scalar.copy(ft[:], psf[:])

            # ---- upsample + blend, in-place into fr_t, then store ----
            for ci in range(NCHUNK):
                for nb in range(HW // 512):
                    off = ci * HW + nb * 512
                    pso = ps_out.tile([128, 512], F32, tag="pso")
                    nc.tensor.matmul(
                        pso[:],
                        ft[:, ci * 128:(ci + 1) * 128],
                        u2a_s[:, nb * 512:(nb + 1) * 512],
                        start=True, stop=False)
                    nc.tensor.matmul(
                        pso[:],
                        idsc_s[:],
                        fr_t[:, off:off + 512],
                        start=False, stop=True)
                    nc.scalar.copy(fr_t[:, off:off + 512], pso[:])
            nc.sync.dma_start(out[b].transpose([1, 0, 2]),
                              fr_t.rearrange("p (a b) -> p a b", a=NCHUNK))

    nc.compile()
    return nc


def _prepare_in_maps(F_rgb, F_d, Wq, bq, Wk, bk, Wv, bv, alpha):
    if "U" not in _CACHE:
        _CACHE["U"] = _bilinear_up_matrix(H, AS)
    U = _CACHE["U"]

    F_rgb = np.ascontiguousarray(np.asarray(F_rgb, np.float32))
    F_d = np.ascontiguousarray(np.asarray(F_d, np.float32))
    a = float(np.asarray(alpha))

    frgb_sh = F_rgb.reshape(NCORES, BPC, NCHUNK, 128, HW)
    fd_sh = F_d.reshape(NCORES, BPC, NCHUNK, 128, HW)

    def wfold(Wx):
        # [c, o] chunks of (Wx / 64)^T
        return np.ascontiguousarray(
            (np.asarray(Wx, np.float64).T / (AS * AS)).reshape(NCHUNK, 128, C)
        ).astype(np.float32)

    wqt = wfold(Wq)
    wkt = wfold(Wk)
    wvt = wfold(Wv)
    bq2 = np.ascontiguousarray(np.asarray(bq, np.float32).reshape(NCHUNK, 128).T)
    bk2 = np.ascontiguousarray(np.asarray(bk, np.float32).reshape(NCHUNK, 128).T)
    bvr = np.asarray(bv, np.float32).reshape(1, C)
    u2a = (a * np.kron(U.T, U.T)).astype(np.float32)
    idsc = ((1.0 - a) * np.eye(128, dtype=np.float64)).astype(np.float32)
    id64 = np.eye(T, dtype=np.float32)
    ones64 = np.ones((1, T), np.float32)

    in_maps = []
    for i in range(NCORES):
        in_maps.append({
            "frgb": np.ascontiguousarray(frgb_sh[i]),
            "fd": np.ascontiguousarray(fd_sh[i]),
            "wqt": wqt, "wkt": wkt, "wvt": wvt,
            "bq2": bq2, "bk2": bk2, "bvr": bvr,
            "u2a": u2a, "idsc": idsc, "id64": id64, "ones64": ones64,
        })
    return in_maps


def _execute(in_maps, **kwargs):
    if "nc" not in _CACHE:
        _CACHE["nc"] = _build_program()
    res = run_bass_kernel_spmd(_CACHE["nc"], in_maps, list(range(NCORES)),
                               **kwargs)
    parts = [res.results[i]["out"].reshape(BPC, C, H, W) for i in range(NCORES)]
    return np.concatenate(parts, axis=0), res


def kernel(F_rgb, F_d, Wq, bq, Wk, bk, Wv, bv, alpha):
    in_maps = _prepare_in_maps(F_rgb, F_d, Wq, bq, Wk, bk, Wv, bv, alpha)
    out, _ = _execute(in_maps)
    return out


# revision 8
# speedup vs baseline: 1.3176x; 1.3176x over previous
"""CrossModalAttention Trainium2 kernel.

Full inputs in, full outputs out; internally sharded data-parallel over the
batch dim across 8 NeuronCores (4 batch items per core).

Per batch item (C=256, H=W=64, AS=8, T=64):
  - F_rgb / F_d are DMA-loaded with an on-the-fly fp32->bf16 cast (SWDGE).
  - avgpool 64x64 -> 8x8 as bf16 pairwise-add trees on DVE (2x perf mode);
    the 1/64 mean factor is folded into the host-prepared weights.
  - Q = Wq@R+bq, K = Wk@D+bk as [o, s] (PE bf16 + ACT bias, bf16 out)
  - VfT = D^T @ Wv^T + ones^T@bv as [s, o] (PE bf16)
  - A = Qf^T Kf [t, s] fp32 PSUM; softmax rows (DVE + ACT exp)
  - AsmT via PE transpose; FattT = AsmT @ VfT [t, c] (PE bf16)
  - out = alpha*up8(Fatt) + (1-alpha)*F_rgb as one accumulating PSUM group
    of two bf16 matmuls per 512-wide block:
       psum  = FattT_chunk^T @ (alpha*kron(U^T,U^T))_block   (upsample)
       psum += ((1-alpha)*I)^T @ F_rgb_block                 (blend)
    ACT copies psum -> fp32 out tile -> one 4MB DMA store per batch item.
"""

import numpy as np
from contextlib import ExitStack

import ml_dtypes
import concourse.bacc as bacc
import concourse.mybir as mybir
import concourse.tile as tile
from concourse.bass_utils import run_bass_kernel_spmd

B, C, H, W = 32, 256, 64, 64
AS = 8
T = AS * AS          # 64 pooled pixels
HW = H * W           # 4096
NCORES = 8
BPC = B // NCORES    # 4 batch items per core
NCHUNK = C // 128    # 2 channel chunks

F32 = mybir.dt.float32
BF16 = mybir.dt.bfloat16
NPBF16 = ml_dtypes.bfloat16


def _bilinear_up_matrix(n_out: int, n_in: int) -> np.ndarray:
    """U[i, p]: weight of coarse pixel p for fine pixel i; half-pixel centers
    with edge clamping (identical to jax.image.resize bilinear upsample)."""
    U = np.zeros((n_out, n_in), np.float64)
    scale = n_in / n_out
    for i in range(n_out):
        src = (i + 0.5) * scale - 0.5
        p0 = int(np.floor(src))
        f = src - p0
        for p, wgt in ((p0, 1.0 - f), (p0 + 1, f)):
            pc = min(max(p, 0), n_in - 1)
            U[i, pc] += wgt
    return U


_CACHE = {}


def _pool_tree(nc, scr_pool, x, dst):
    """Sum 8x8 blocks: x = [128, HW] bf16 view (h*64+w), dst = [128, 64]."""
    xv = x.rearrange("p (s u) -> p s u", u=AS)            # [128, 512, 8]
    w1 = scr_pool.tile([128, 2048], BF16, tag="w1")
    w1v = w1.rearrange("p (s u) -> p s u", u=4)
    nc.vector.tensor_add(w1v, xv[:, :, 0:4], xv[:, :, 4:8])
    w2 = scr_pool.tile([128, 1024], BF16, tag="w2")
    w2v = w2.rearrange("p (s u) -> p s u", u=2)
    nc.vector.tensor_add(w2v, w1v[:, :, 0:2], w1v[:, :, 2:4])
    t3 = scr_pool.tile([128, 512], BF16, tag="t3")
    t3v = t3.rearrange("p (s u) -> p s u", u=1)
    nc.vector.tensor_add(t3v, w2v[:, :, 0:1], w2v[:, :, 1:2])
    # t3 free index = h*8 + wq, h = hp*8 + v -> view [hp, v, wq]
    hv = t3.rearrange("p (a v w) -> p a v w", a=AS, v=AS)
    h1 = scr_pool.tile([128, 256], BF16, tag="h1")
    h1v = h1.rearrange("p (a v w) -> p a v w", a=AS, v=4)
    nc.vector.tensor_add(h1v, hv[:, :, 0:4, :], hv[:, :, 4:8, :])
    h2 = scr_pool.tile([128, 128], BF16, tag="h2")
    h2v = h2.rearrange("p (a v w) -> p a v w", a=AS, v=2)
    nc.vector.tensor_add(h2v, h1v[:, :, 0:2, :], h1v[:, :, 2:4, :])
    dv = dst.rearrange("p (a v w) -> p a v w", a=AS, v=1)
    nc.vector.tensor_add(dv, h2v[:, :, 0:1, :], h2v[:, :, 1:2, :])


def _build_program():
    nc = bacc.Bacc("TRN2", target_bir_lowering=False, debug=False,
                   num_devices=NCORES)

    frgb = nc.dram_tensor("frgb", [BPC, NCHUNK, 128, HW], F32,
                          kind="ExternalInput").ap()
    fd = nc.dram_tensor("fd", [BPC, NCHUNK, 128, HW], F32,
                        kind="ExternalInput").ap()
    wqt = nc.dram_tensor("wqt", [NCHUNK, 128, C], BF16, kind="ExternalInput").ap()
    wkt = nc.dram_tensor("wkt", [NCHUNK, 128, C], BF16, kind="ExternalInput").ap()
    wvt = nc.dram_tensor("wvt", [NCHUNK, 128, C], BF16, kind="ExternalInput").ap()
    bq2 = nc.dram_tensor("bq2", [128, NCHUNK], F32, kind="ExternalInput").ap()
    bk2 = nc.dram_tensor("bk2", [128, NCHUNK], F32, kind="ExternalInput").ap()
    bvr = nc.dram_tensor("bvr", [1, C], BF16, kind="ExternalInput").ap()
    u2a = nc.dram_tensor("u2a", [T, HW], BF16, kind="ExternalInput").ap()
    idsc = nc.dram_tensor("idsc", [128, 128], BF16, kind="ExternalInput").ap()
    id64 = nc.dram_tensor("id64", [T, T], BF16, kind="ExternalInput").ap()
    ones64 = nc.dram_tensor("ones64", [1, T], BF16, kind="ExternalInput").ap()
    out = nc.dram_tensor("out", [BPC, NCHUNK, 128, HW], F32,
                         kind="ExternalOutput").ap()

    with tile.TileContext(nc) as tc, ExitStack() as ctx:
        consts = ctx.enter_context(tc.tile_pool(name="consts", bufs=1))
        fr_pool = ctx.enter_context(tc.tile_pool(name="fr", bufs=2))
        fd_pool = ctx.enter_context(tc.tile_pool(name="fdp", bufs=2))
        out_pool = ctx.enter_context(tc.tile_pool(name="outp", bufs=2))
        scr_pool = ctx.enter_context(tc.tile_pool(name="scr", bufs=2))
        small = ctx.enter_context(tc.tile_pool(name="small", bufs=2))
        ps_small = ctx.enter_context(
            tc.tile_pool(name="pss", bufs=3, space="PSUM"))
        ps_out = ctx.enter_context(
            tc.tile_pool(name="pso", bufs=4, space="PSUM"))

        # ---- constants into SBUF ----
        wqt_s = consts.tile([128, NCHUNK * C], BF16)   # [c, (ci, o)]
        nc.sync.dma_start(wqt_s.rearrange("p (a b) -> p a b", a=NCHUNK),
                          wqt.transpose([1, 0, 2]))
        wkt_s = consts.tile([128, NCHUNK * C], BF16)
        nc.sync.dma_start(wkt_s.rearrange("p (a b) -> p a b", a=NCHUNK),
                          wkt.transpose([1, 0, 2]))
        wvt_s = consts.tile([128, NCHUNK * C], BF16)
        nc.sync.dma_start(wvt_s.rearrange("p (a b) -> p a b", a=NCHUNK),
                          wvt.transpose([1, 0, 2]))
        bq_s = consts.tile([128, NCHUNK], F32)
        nc.sync.dma_start(bq_s[:], bq2[:])
        bk_s = consts.tile([128, NCHUNK], F32)
        nc.sync.dma_start(bk_s[:], bk2[:])
        bvr_s = consts.tile([1, C], BF16)
        nc.sync.dma_start(bvr_s[:], bvr[:])
        u2a_s = consts.tile([T, HW], BF16)
        nc.sync.dma_start(u2a_s[:], u2a[:])
        idsc_s = consts.tile([128, 128], BF16)
        nc.sync.dma_start(idsc_s[:], idsc[:])
        id64_s = consts.tile([T, T], BF16)
        nc.sync.dma_start(id64_s[:], id64[:])
        ones_s = consts.tile([1, T], BF16)
        nc.sync.dma_start(ones_s[:], ones64[:])

        for b in range(BPC):
            # ---- load F_rgb / F_d with fp32 -> bf16 cast (SWDGE) ----
            fr_t = fr_pool.tile([128, NCHUNK * HW], BF16, tag="fr")
            nc.gpsimd.dma_start(fr_t.rearrange("p (a b) -> p a b", a=NCHUNK),
                                frgb[b].transpose([1, 0, 2]))
            fd_t = fd_pool.tile([128, NCHUNK * HW], BF16, tag="fd")
            nc.gpsimd.dma_start(fd_t.rearrange("p (a b) -> p a b", a=NCHUNK),
                                fd[b].transpose([1, 0, 2]))

            # ---- avgpool sum trees (bf16) ----
            rs_t = small.tile([128, NCHUNK * T], BF16, tag="rs")
            ds_t = small.tile([128, NCHUNK * T], BF16, tag="ds")
            for src_t, dst_t in ((fr_t, rs_t), (fd_t, ds_t)):
                for ci in range(NCHUNK):
                    _pool_tree(nc, scr_pool,
                               src_t[:, ci * HW:(ci + 1) * HW],
                               dst_t[:, ci * T:(ci + 1) * T])

            # ---- Q, K: [o, s] with per-partition bias (bf16 out) ----
            qf_t = small.tile([128, NCHUNK * T], BF16, tag="qf")
            kf_t = small.tile([128, NCHUNK * T], BF16, tag="kf")
            for w_s, b_s, sums, dst in ((wqt_s, bq_s, rs_t, qf_t),
                                        (wkt_s, bk_s, ds_t, kf_t)):
                for oj in range(NCHUNK):
                    psq = ps_small.tile([128, T], F32, tag="pss")
                    for ci in range(NCHUNK):
                        nc.tensor.matmul(
                            psq[:],
                            w_s[:, ci * C + oj * 128: ci * C + (oj + 1) * 128],
                            sums[:, ci * T:(ci + 1) * T],
                            start=(ci == 0), stop=(ci == NCHUNK - 1))
                    nc.scalar.activation(
                        dst[:, oj * T:(oj + 1) * T], psq[:],
                        mybir.ActivationFunctionType.Identity,
                        bias=b_s[:, oj:oj + 1], scale=1.0)

            # ---- VfT = D^T Wv^T + ones^T bv : [s, o] ----
            psv = ps_small.tile([T, C], F32, tag="pss")
            for ci in range(NCHUNK):
                nc.tensor.matmul(psv[:],
                                 ds_t[:, ci * T:(ci + 1) * T],
                                 wvt_s[:, ci * C:(ci + 1) * C],
                                 start=(ci == 0), stop=False)
            nc.tensor.matmul(psv[:], ones_s[:], bvr_s[:], start=False, stop=True)
            vft = small.tile([T, C], BF16, tag="vft")
            nc.scalar.copy(vft[:], psv[:])

            # ---- A = Qf^T Kf : [t, s] ----
            psa = ps_small.tile([T, T], F32, tag="pss")
            for oj in range(NCHUNK):
                nc.tensor.matmul(psa[:],
                                 qf_t[:, oj * T:(oj + 1) * T],
                                 kf_t[:, oj * T:(oj + 1) * T],
                                 start=(oj == 0), stop=(oj == NCHUNK - 1))

            # ---- softmax over free dim ----
            negmax = small.tile([T, 1], F32, tag="negmax")
            nc.vector.tensor_reduce(negmax[:], psa[:],
                                    axis=mybir.AxisListType.X,
                                    op=mybir.AluOpType.max, negate=True)
            e_t = small.tile([T, T], F32, tag="e")
            nc.scalar.activation(e_t[:], psa[:],
                                 mybir.ActivationFunctionType.Exp,
                                 bias=negmax[:, 0:1], scale=1.0)
            s1 = small.tile([T, 1], F32, tag="s1")
            nc.vector.reduce_sum(s1[:], e_t[:], axis=mybir.AxisListType.X)
            r1 = small.tile([T, 1], F32, tag="r1")
            nc.vector.reciprocal(r1[:], s1[:])
            asm = small.tile([T, T], BF16, tag="asm")
            nc.scalar.mul(asm[:], e_t[:], r1[:, 0:1])

            # ---- AsmT via PE transpose ----
            psat = ps_small.tile([T, T], BF16, tag="psst", bufs=1)
            nc.tensor.transpose(psat[:], asm[:], id64_s[:])
            asmt = small.tile([T, T], BF16, tag="asmt")
            nc.scalar.copy(asmt[:], psat[:])

            # ---- FattT = AsmT @ VfT : [t, c] ----
            psf = ps_small.tile([T, C], F32, tag="pss")
            nc.tensor.matmul(psf[:], asmt[:], vft[:], start=True, stop=True)
            ft = small.tile([T, C], BF16, tag="ft")
            nc.scalar.copy(ft[:], psf[:])

            # ---- upsample + blend -> fp32 out tile, then store ----
            out_t = out_pool.tile([128, NCHUNK * HW], F32, tag="ot")
            for ci in range(NCHUNK):
                for nb in range(HW // 512):
                    off = ci * HW + nb * 512
                    pso = ps_out.tile([128, 512], F32, tag="pso")
                    nc.tensor.matmul(
                        pso[:],
                        ft[:, ci * 128:(ci + 1) * 128],
                        u2a_s[:, nb * 512:(nb + 1) * 512],
                        start=True, stop=False)
                    nc.tensor.matmul(
                        pso[:],
                        idsc_s[:],
                        fr_t[:, off:off + 512],
                        start=False, stop=True)
                    nc.scalar.copy(out_t[:, off:off + 512], pso[:])
            nc.sync.dma_start(out[b].transpose([1, 0, 2]),
                              out_t.rearrange("p (a b) -> p a b", a=NCHUNK))

    nc.compile()
    return nc


def _prepare_in_maps(F_rgb, F_d, Wq, bq, Wk, bk, Wv, bv, alpha):
    if "U" not in _CACHE:
        _CACHE["U"] = _bilinear_up_matrix(H, AS)
    U = _CACHE["U"]

    F_rgb = np.ascontiguousarray(np.asarray(F_rgb, np.float32))
    F_d = np.ascontiguousarray(np.asarray(F_d, np.float32))
    a = float(np.asarray(alpha))

    frgb_sh = F_rgb.reshape(NCORES, BPC, NCHUNK, 128, HW)
    fd_sh = F_d.reshape(NCORES, BPC, NCHUNK, 128, HW)

    def wfold(Wx):
        # [c, o] chunks of (Wx / 64)^T
        return np.ascontiguousarray(
            (np.asarray(Wx, np.float64).T / (AS * AS)).reshape(NCHUNK, 128, C)
        ).astype(NPBF16)

    wqt = wfold(Wq)
    wkt = wfold(Wk)
    wvt = wfold(Wv)
    bq2 = np.ascontiguousarray(np.asarray(bq, np.float32).reshape(NCHUNK, 128).T)
    bk2 = np.ascontiguousarray(np.asarray(bk, np.float32).reshape(NCHUNK, 128).T)
    bvr = np.asarray(bv, np.float32).reshape(1, C).astype(NPBF16)
    u2a = (a * np.kron(U.T, U.T)).astype(NPBF16)
    idsc = ((1.0 - a) * np.eye(128, dtype=np.float64)).astype(NPBF16)
    id64 = np.eye(T, dtype=np.float32).astype(NPBF16)
    ones64 = np.ones((1, T), NPBF16)

    in_maps = []
    for i in range(NCORES):
        in_maps.append({
            "frgb": np.ascontiguousarray(frgb_sh[i]),
            "fd": np.ascontiguousarray(fd_sh[i]),
            "wqt": wqt, "wkt": wkt, "wvt": wvt,
            "bq2": bq2, "bk2": bk2, "bvr": bvr,
            "u2a": u2a, "idsc": idsc, "id64": id64, "ones64": ones64,
        })
    return in_maps


def _execute(in_maps, **kwargs):
    if "nc" not in _CACHE:
        _CACHE["nc"] = _build_program()
    res = run_bass_kernel_spmd(_CACHE["nc"], in_maps, list(range(NCORES)),
                               **kwargs)
    parts = [res.results[i]["out"].reshape(BPC, C, H, W) for i in range(NCORES)]
    return np.concatenate(parts, axis=0), res


def kernel(F_rgb, F_d, Wq, bq, Wk, bk, Wv, bv, alpha):
    in_maps = _prepare_in_maps(F_rgb, F_d, Wq, bq, Wk, bk, Wv, bv, alpha)
    out, _ = _execute(in_maps)
    return out


# revision 9
# speedup vs baseline: 1.8810x; 1.4276x over previous
"""CrossModalAttention Trainium2 kernel.

Full inputs in, full outputs out; internally sharded data-parallel over the
batch dim across 8 NeuronCores (4 batch items per core).

Per batch item (C=256, H=W=64, AS=8, T=64):
  - F_rgb / F_d are DMA-loaded with an on-the-fly fp32->bf16 cast (SWDGE).
  - avgpool 64x64 -> 8x8 as bf16 pairwise-add trees on DVE (2x perf mode);
    the 1/64 mean factor is folded into the host-prepared weights.
  - Q = Wq@R+bq, K = Wk@D+bk as [o, s] (PE bf16 + ACT bias, bf16 out)
  - VfT = D^T @ Wv^T + ones^T@bv as [s, o] (PE bf16)
  - A = Qf^T Kf [t, s] fp32 PSUM; softmax rows (DVE + ACT exp)
  - AsmT via PE transpose; FattT = AsmT @ VfT [t, c] (PE bf16)
  - out = alpha*up8(Fatt) + (1-alpha)*F_rgb as one accumulating PSUM group
    of two bf16 matmuls per 512-wide block:
       psum  = FattT_chunk^T @ (alpha*kron(U^T,U^T))_block   (upsample)
       psum += ((1-alpha)*I)^T @ F_rgb_block                 (blend)
    ACT copies psum -> fp32 out tile -> one 4MB DMA store per batch item.
"""

import numpy as np
from contextlib import ExitStack

import ml_dtypes
import concourse.bacc as bacc
import concourse.mybir as mybir
import concourse.tile as tile
from concourse.bass_utils import run_bass_kernel_spmd

B, C, H, W = 32, 256, 64, 64
AS = 8
T = AS * AS          # 64 pooled pixels
HW = H * W           # 4096
NCORES = 8
BPC = B // NCORES    # 4 batch items per core
NCHUNK = C // 128    # 2 channel chunks

F32 = mybir.dt.float32
BF16 = mybir.dt.bfloat16
NPBF16 = ml_dtypes.bfloat16


def _bilinear_up_matrix(n_out: int, n_in: int) -> np.ndarray:
    """U[i, p]: weight of coarse pixel p for fine pixel i; half-pixel centers
    with edge clamping (identical to jax.image.resize bilinear upsample)."""
    U = np.zeros((n_out, n_in), np.float64)
    scale = n_in / n_out
    for i in range(n_out):
        src = (i + 0.5) * scale - 0.5
        p0 = int(np.floor(src))
        f = src - p0
        for p, wgt in ((p0, 1.0 - f), (p0 + 1, f)):
            pc = min(max(p, 0), n_in - 1)
            U[i, pc] += wgt
    return U


_CACHE = {}


def _pool_tree(nc, scr_pool, x, dst):
    """Sum 8x8 blocks: x = [128, HW] bf16 view (h*64+w), dst = [128, 64]."""
    xv = x.rearrange("p (s u) -> p s u", u=AS)            # [128, 512, 8]
    w1 = scr_pool.tile([128, 2048], BF16, tag="w1")
    w1v = w1.rearrange("p (s u) -> p s u", u=4)
    nc.vector.tensor_add(w1v, xv[:, :, 0:4], xv[:, :, 4:8])
    w2 = scr_pool.tile([128, 1024], BF16, tag="w2")
    w2v = w2.rearrange("p (s u) -> p s u", u=2)
    nc.vector.tensor_add(w2v, w1v[:, :, 0:2], w1v[:, :, 2:4])
    t3 = scr_pool.tile([128, 512], BF16, tag="t3")
    t3v = t3.rearrange("p (s u) -> p s u", u=1)
    nc.vector.tensor_add(t3v, w2v[:, :, 0:1], w2v[:, :, 1:2])
    # t3 free index = h*8 + wq, h = hp*8 + v -> view [hp, v, wq]
    hv = t3.rearrange("p (a v w) -> p a v w", a=AS, v=AS)
    h1 = scr_pool.tile([128, 256], BF16, tag="h1")
    h1v = h1.rearrange("p (a v w) -> p a v w", a=AS, v=4)
    nc.vector.tensor_add(h1v, hv[:, :, 0:4, :], hv[:, :, 4:8, :])
    h2 = scr_pool.tile([128, 128], BF16, tag="h2")
    h2v = h2.rearrange("p (a v w) -> p a v w", a=AS, v=2)
    nc.vector.tensor_add(h2v, h1v[:, :, 0:2, :], h1v[:, :, 2:4, :])
    dv = dst.rearrange("p (a v w) -> p a v w", a=AS, v=1)
    nc.vector.tensor_add(dv, h2v[:, :, 0:1, :], h2v[:, :, 1:2, :])


def _build_program():
    nc = bacc.Bacc("TRN2", target_bir_lowering=False, debug=False,
                   num_devices=NCORES)

    frgb = nc.dram_tensor("frgb", [BPC, NCHUNK, 128, HW], BF16,
                          kind="ExternalInput").ap()
    fd = nc.dram_tensor("fd", [BPC, NCHUNK, 128, HW], BF16,
                        kind="ExternalInput").ap()
    wqt = nc.dram_tensor("wqt", [NCHUNK, 128, C], BF16, kind="ExternalInput").ap()
    wkt = nc.dram_tensor("wkt", [NCHUNK, 128, C], BF16, kind="ExternalInput").ap()
    wvt = nc.dram_tensor("wvt", [NCHUNK, 128, C], BF16, kind="ExternalInput").ap()
    bq2 = nc.dram_tensor("bq2", [128, NCHUNK], F32, kind="ExternalInput").ap()
    bk2 = nc.dram_tensor("bk2", [128, NCHUNK], F32, kind="ExternalInput").ap()
    bvr = nc.dram_tensor("bvr", [1, C], BF16, kind="ExternalInput").ap()
    u2a = nc.dram_tensor("u2a", [T, HW], BF16, kind="ExternalInput").ap()
    idsc = nc.dram_tensor("idsc", [128, 128], BF16, kind="ExternalInput").ap()
    id64 = nc.dram_tensor("id64", [T, T], BF16, kind="ExternalInput").ap()
    ones64 = nc.dram_tensor("ones64", [1, T], BF16, kind="ExternalInput").ap()
    out = nc.dram_tensor("out", [BPC, NCHUNK, 128, HW], BF16,
                         kind="ExternalOutput").ap()

    with tile.TileContext(nc) as tc, ExitStack() as ctx:
        consts = ctx.enter_context(tc.tile_pool(name="consts", bufs=1))
        fr_pool = ctx.enter_context(tc.tile_pool(name="fr", bufs=3))
        fd_pool = ctx.enter_context(tc.tile_pool(name="fdp", bufs=3))
        out_pool = ctx.enter_context(tc.tile_pool(name="outp", bufs=2))
        scr_pool = ctx.enter_context(tc.tile_pool(name="scr", bufs=2))
        small = ctx.enter_context(tc.tile_pool(name="small", bufs=2))
        ps_small = ctx.enter_context(
            tc.tile_pool(name="pss", bufs=3, space="PSUM"))
        ps_out = ctx.enter_context(
            tc.tile_pool(name="pso", bufs=4, space="PSUM"))

        # ---- constants into SBUF ----
        wqt_s = consts.tile([128, NCHUNK * C], BF16)   # [c, (ci, o)]
        nc.sync.dma_start(wqt_s.rearrange("p (a b) -> p a b", a=NCHUNK),
                          wqt.transpose([1, 0, 2]))
        wkt_s = consts.tile([128, NCHUNK * C], BF16)
        nc.sync.dma_start(wkt_s.rearrange("p (a b) -> p a b", a=NCHUNK),
                          wkt.transpose([1, 0, 2]))
        wvt_s = consts.tile([128, NCHUNK * C], BF16)
        nc.sync.dma_start(wvt_s.rearrange("p (a b) -> p a b", a=NCHUNK),
                          wvt.transpose([1, 0, 2]))
        bq_s = consts.tile([128, NCHUNK], F32)
        nc.sync.dma_start(bq_s[:], bq2[:])
        bk_s = consts.tile([128, NCHUNK], F32)
        nc.sync.dma_start(bk_s[:], bk2[:])
        bvr_s = consts.tile([1, C], BF16)
        nc.sync.dma_start(bvr_s[:], bvr[:])
        u2a_s = consts.tile([T, HW], BF16)
        nc.sync.dma_start(u2a_s[:], u2a[:])
        idsc_s = consts.tile([128, 128], BF16)
        nc.sync.dma_start(idsc_s[:], idsc[:])
        id64_s = consts.tile([T, T], BF16)
        nc.sync.dma_start(id64_s[:], id64[:])
        ones_s = consts.tile([1, T], BF16)
        nc.sync.dma_start(ones_s[:], ones64[:])

        for b in range(BPC):
            # ---- load F_rgb / F_d with fp32 -> bf16 cast (SWDGE) ----
            fr_t = fr_pool.tile([128, NCHUNK * HW], BF16, tag="fr")
            nc.sync.dma_start(fr_t.rearrange("p (a b) -> p a b", a=NCHUNK),
                              frgb[b].transpose([1, 0, 2]))
            fd_t = fd_pool.tile([128, NCHUNK * HW], BF16, tag="fd")
            nc.sync.dma_start(fd_t.rearrange("p (a b) -> p a b", a=NCHUNK),
                              fd[b].transpose([1, 0, 2]))

            # ---- avgpool sum trees (bf16) ----
            rs_t = small.tile([128, NCHUNK * T], BF16, tag="rs")
            ds_t = small.tile([128, NCHUNK * T], BF16, tag="ds")
            for src_t, dst_t in ((fr_t, rs_t), (fd_t, ds_t)):
                for ci in range(NCHUNK):
                    _pool_tree(nc, scr_pool,
                               src_t[:, ci * HW:(ci + 1) * HW],
                               dst_t[:, ci * T:(ci + 1) * T])

            # ---- Q, K: [o, s] with per-partition bias (bf16 out) ----
            qf_t = small.tile([128, NCHUNK * T], BF16, tag="qf")
            kf_t = small.tile([128, NCHUNK * T], BF16, tag="kf")
            for w_s, b_s, sums, dst in ((wqt_s, bq_s, rs_t, qf_t),
                                        (wkt_s, bk_s, ds_t, kf_t)):
                for oj in range(NCHUNK):
                    psq = ps_small.tile([128, T], F32, tag="pss")
                    for ci in range(NCHUNK):
                        nc.tensor.matmul(
                            psq[:],
                            w_s[:, ci * C + oj * 128: ci * C + (oj + 1) * 128],
                            sums[:, ci * T:(ci + 1) * T],
                            start=(ci == 0), stop=(ci == NCHUNK - 1))
                    nc.scalar.activation(
                        dst[:, oj * T:(oj + 1) * T], psq[:],
                        mybir.ActivationFunctionType.Identity,
                        bias=b_s[:, oj:oj + 1], scale=1.0)

            # ---- VfT = D^T Wv^T + ones^T bv : [s, o] ----
            psv = ps_small.tile([T, C], F32, tag="pss")
            for ci in range(NCHUNK):
                nc.tensor.matmul(psv[:],
                                 ds_t[:, ci * T:(ci + 1) * T],
                                 wvt_s[:, ci * C:(ci + 1) * C],
                                 start=(ci == 0), stop=False)
            nc.tensor.matmul(psv[:], ones_s[:], bvr_s[:], start=False, stop=True)
            vft = small.tile([T, C], BF16, tag="vft")
            nc.scalar.copy(vft[:], psv[:])

            # ---- A = Qf^T Kf : [t, s] ----
            psa = ps_small.tile([T, T], F32, tag="pss")
            for oj in range(NCHUNK):
                nc.tensor.matmul(psa[:],
                                 qf_t[:, oj * T:(oj + 1) * T],
                                 kf_t[:, oj * T:(oj + 1) * T],
                                 start=(oj == 0), stop=(oj == NCHUNK - 1))

            # ---- softmax over free dim ----
            negmax = small.tile([T, 1], F32, tag="negmax")
            nc.vector.tensor_reduce(negmax[:], psa[:],
                                    axis=mybir.AxisListType.X,
                                    op=mybir.AluOpType.max, negate=True)
            e_t = small.tile([T, T], F32, tag="e")
            nc.scalar.activation(e_t[:], psa[:],
                                 mybir.ActivationFunctionType.Exp,
                                 bias=negmax[:, 0:1], scale=1.0)
            s1 = small.tile([T, 1], F32, tag="s1")
            nc.vector.reduce_sum(s1[:], e_t[:], axis=mybir.AxisListType.X)
            r1 = small.tile([T, 1], F32, tag="r1")
            nc.vector.reciprocal(r1[:], s1[:])
            asm = small.tile([T, T], BF16, tag="asm")
            nc.scalar.mul(asm[:], e_t[:], r1[:, 0:1])

            # ---- AsmT via PE transpose ----
            psat = ps_small.tile([T, T], BF16, tag="psst", bufs=1)
            nc.tensor.transpose(psat[:], asm[:], id64_s[:])
            asmt = small.tile([T, T], BF16, tag="asmt")
            nc.scalar.copy(asmt[:], psat[:])

            # ---- FattT = AsmT @ VfT : [t, c] ----
            psf = ps_small.tile([T, C], F32, tag="pss")
            nc.tensor.matmul(psf[:], asmt[:], vft[:], start=True, stop=True)
            ft = small.tile([T, C], BF16, tag="ft")
            nc.scalar.copy(ft[:], psf[:])

            # ---- upsample + blend -> fp32 out tile, then store ----
            out_t = out_pool.tile([128, NCHUNK * HW], BF16, tag="ot")
            for ci in range(NCHUNK):
                for nb in range(HW // 512):
                    off = ci * HW + nb * 512
                    pso = ps_out.tile([128, 512], F32, tag="pso")
                    nc.tensor.matmul(
                        pso[:],
                        ft[:, ci * 128:(ci + 1) * 128],
                        u2a_s[:, nb * 512:(nb + 1) * 512],
                        start=True, stop=False)
                    nc.tensor.matmul(
                        pso[:],
                        idsc_s[:],
                        fr_t[:, off:off + 512],
                        start=False, stop=True)
                    nc.scalar.copy(out_t[:, off:off + 512], pso[:])
            nc.sync.dma_start(out[b].transpose([1, 0, 2]),
                              out_t.rearrange("p (a b) -> p a b", a=NCHUNK))

    nc.compile()
    return nc


def _prepare_in_maps(F_rgb, F_d, Wq, bq, Wk, bk, Wv, bv, alpha):
    if "U" not in _CACHE:
        _CACHE["U"] = _bilinear_up_matrix(H, AS)
    U = _CACHE["U"]

    F_rgb = np.asarray(F_rgb, np.float32).astype(NPBF16)
    F_d = np.asarray(F_d, np.float32).astype(NPBF16)
    a = float(np.asarray(alpha))

    frgb_sh = F_rgb.reshape(NCORES, BPC, NCHUNK, 128, HW)
    fd_sh = F_d.reshape(NCORES, BPC, NCHUNK, 128, HW)

    def wfold(Wx):
        # [c, o] chunks of (Wx / 64)^T
        return np.ascontiguousarray(
            (np.asarray(Wx, np.float64).T / (AS * AS)).reshape(NCHUNK, 128, C)
        ).astype(NPBF16)

    wqt = wfold(Wq)
    wkt = wfold(Wk)
    wvt = wfold(Wv)
    bq2 = np.ascontiguousarray(np.asarray(bq, np.float32).reshape(NCHUNK, 128).T)
    bk2 = np.ascontiguousarray(np.asarray(bk, np.float32).reshape(NCHUNK, 128).T)
    bvr = np.asarray(bv, np.float32).reshape(1, C).astype(NPBF16)
    u2a = (a * np.kron(U.T, U.T)).astype(NPBF16)
    idsc = ((1.0 - a) * np.eye(128, dtype=np.float64)).astype(NPBF16)
    id64 = np.eye(T, dtype=np.float32).astype(NPBF16)
    ones64 = np.ones((1, T), NPBF16)

    in_maps = []
    for i in range(NCORES):
        in_maps.append({
            "frgb": np.ascontiguousarray(frgb_sh[i]),
            "fd": np.ascontiguousarray(fd_sh[i]),
            "wqt": wqt, "wkt": wkt, "wvt": wvt,
            "bq2": bq2, "bk2": bk2, "bvr": bvr,
            "u2a": u2a, "idsc": idsc, "id64": id64, "ones64": ones64,
        })
    return in_maps


def _execute(in_maps, **kwargs):
    if "nc" not in _CACHE:
        _CACHE["nc"] = _build_program()
    res = run_bass_kernel_spmd(_CACHE["nc"], in_maps, list(range(NCORES)),
                               **kwargs)
    parts = [res.results[i]["out"].astype(np.float32).reshape(BPC, C, H, W)
             for i in range(NCORES)]
    return np.concatenate(parts, axis=0), res


def kernel(F_rgb, F_d, Wq, bq, Wk, bk, Wv, bv, alpha):
    in_maps = _prepare_in_maps(F_rgb, F_d, Wq, bq, Wk, bk, Wv, bv, alpha)
    out, _ = _execute(in_maps)
    return out


# revision 11
# speedup vs baseline: 2.2832x; 1.2138x over previous
"""CrossModalAttention Trainium2 kernel.

Full inputs in, full outputs out; internally sharded data-parallel over the
batch dim across 8 NeuronCores (4 batch items per core).

Per batch item (C=256, H=W=64, AS=8, T=64):
  - Host pre-casts F_d -> fp16 and F_rgb -> (1-alpha)*F_rgb in fp16 (halves
    DMA read bytes; the (1-alpha) blend scale rides the cast for free and is
    unfolded via the Q weights).
  - avgpool 64x64 -> 8x8 as fp16 pairwise-add trees on DVE (2x perf mode);
    the 1/64 mean factor (and 1/(1-alpha) for R) folded into host weights.
  - Q = Wq@R+bq, K = Wk@D+bk as [o, s] (PE fp16 + ACT bias, fp16 out)
  - VfT = D^T @ Wv^T + ones^T@bv as [s, o] (PE fp16)
  - A = Qf^T Kf [t, s] fp32 PSUM; softmax rows (DVE + ACT exp, fp32)
  - AsmT via PE transpose; FattT = AsmT @ VfT [t, c] (PE fp16)
  - upsample: psum = FattT_chunk^T @ (alpha*kron(U^T,U^T))_block per
    512-wide block (PE fp16); ACT copies psum -> fp16 out tile; DVE adds the
    pre-scaled F_rgb block at fp16 2x rate (the blend); one DMA store per
    batch item; host upcasts to fp32.
"""

import numpy as np
from contextlib import ExitStack

import concourse.bacc as bacc
import concourse.mybir as mybir
import concourse.tile as tile
from concourse.bass_utils import run_bass_kernel_spmd

B, C, H, W = 32, 256, 64, 64
AS = 8
T = AS * AS          # 64 pooled pixels
HW = H * W           # 4096
NCORES = 8
BPC = B // NCORES    # 4 batch items per core
NCHUNK = C // 128    # 2 channel chunks

F32 = mybir.dt.float32
F16 = mybir.dt.float16
NPF16 = np.float16


def _bilinear_up_matrix(n_out: int, n_in: int) -> np.ndarray:
    """U[i, p]: weight of coarse pixel p for fine pixel i; half-pixel centers
    with edge clamping (identical to jax.image.resize bilinear upsample)."""
    U = np.zeros((n_out, n_in), np.float64)
    scale = n_in / n_out
    for i in range(n_out):
        src = (i + 0.5) * scale - 0.5
        p0 = int(np.floor(src))
        f = src - p0
        for p, wgt in ((p0, 1.0 - f), (p0 + 1, f)):
            pc = min(max(p, 0), n_in - 1)
            U[i, pc] += wgt
    return U


_CACHE = {}


def _pool_tree(nc, scr_pool, x, dst):
    """Sum 8x8 blocks: x = [128, HW] fp16 view (h*64+w), dst = [128, 64]."""
    xv = x.rearrange("p (s u) -> p s u", u=AS)            # [128, 512, 8]
    w1 = scr_pool.tile([128, 2048], F16, tag="w1")
    w1v = w1.rearrange("p (s u) -> p s u", u=4)
    nc.vector.tensor_add(w1v, xv[:, :, 0:4], xv[:, :, 4:8])
    w2 = scr_pool.tile([128, 1024], F16, tag="w2")
    w2v = w2.rearrange("p (s u) -> p s u", u=2)
    nc.vector.tensor_add(w2v, w1v[:, :, 0:2], w1v[:, :, 2:4])
    t3 = scr_pool.tile([128, 512], F16, tag="t3")
    t3v = t3.rearrange("p (s u) -> p s u", u=1)
    nc.vector.tensor_add(t3v, w2v[:, :, 0:1], w2v[:, :, 1:2])
    # t3 free index = h*8 + wq, h = hp*8 + v -> view [hp, v, wq]
    hv = t3.rearrange("p (a v w) -> p a v w", a=AS, v=AS)
    h1 = scr_pool.tile([128, 256], F16, tag="h1")
    h1v = h1.rearrange("p (a v w) -> p a v w", a=AS, v=4)
    nc.vector.tensor_add(h1v, hv[:, :, 0:4, :], hv[:, :, 4:8, :])
    h2 = scr_pool.tile([128, 128], F16, tag="h2")
    h2v = h2.rearrange("p (a v w) -> p a v w", a=AS, v=2)
    nc.vector.tensor_add(h2v, h1v[:, :, 0:2, :], h1v[:, :, 2:4, :])
    dv = dst.rearrange("p (a v w) -> p a v w", a=AS, v=1)
    nc.vector.tensor_add(dv, h2v[:, :, 0:1, :], h2v[:, :, 1:2, :])


def _build_program(blend: bool):
    nc = bacc.Bacc("TRN2", target_bir_lowering=False, debug=False,
                   num_devices=NCORES)

    frgb = nc.dram_tensor("frgb", [BPC, NCHUNK, 128, HW], F16,
                          kind="ExternalInput").ap()
    fd = nc.dram_tensor("fd", [BPC, NCHUNK, 128, HW], F16,
                        kind="ExternalInput").ap()
    wqt = nc.dram_tensor("wqt", [NCHUNK, 128, C], F16, kind="ExternalInput").ap()
    wkt = nc.dram_tensor("wkt", [NCHUNK, 128, C], F16, kind="ExternalInput").ap()
    wvt = nc.dram_tensor("wvt", [NCHUNK, 128, C], F16, kind="ExternalInput").ap()
    bq2 = nc.dram_tensor("bq2", [128, NCHUNK], F32, kind="ExternalInput").ap()
    bk2 = nc.dram_tensor("bk2", [128, NCHUNK], F32, kind="ExternalInput").ap()
    bvr = nc.dram_tensor("bvr", [1, C], F16, kind="ExternalInput").ap()
    u2a = nc.dram_tensor("u2a", [T, HW], F16, kind="ExternalInput").ap()
    id64 = nc.dram_tensor("id64", [T, T], F16, kind="ExternalInput").ap()
    ones64 = nc.dram_tensor("ones64", [1, T], F16, kind="ExternalInput").ap()
    out = nc.dram_tensor("out", [BPC, NCHUNK, 128, HW], F16,
                         kind="ExternalOutput").ap()

    with tile.TileContext(nc) as tc, ExitStack() as ctx:
        consts = ctx.enter_context(tc.tile_pool(name="consts", bufs=1))
        fr_pool = ctx.enter_context(tc.tile_pool(name="fr", bufs=3))
        fd_pool = ctx.enter_context(tc.tile_pool(name="fdp", bufs=3))
        out_pool = ctx.enter_context(tc.tile_pool(name="outp", bufs=2))
        scr_pool = ctx.enter_context(tc.tile_pool(name="scr", bufs=2))
        small = ctx.enter_context(tc.tile_pool(name="small", bufs=2))
        ps_small = ctx.enter_context(
            tc.tile_pool(name="pss", bufs=3, space="PSUM"))
        ps_out = ctx.enter_context(
            tc.tile_pool(name="pso", bufs=4, space="PSUM"))

        # ---- constants into SBUF ----
        wqt_s = consts.tile([128, NCHUNK * C], F16)   # [c, (ci, o)]
        nc.sync.dma_start(wqt_s.rearrange("p (a b) -> p a b", a=NCHUNK),
                          wqt.transpose([1, 0, 2]))
        wkt_s = consts.tile([128, NCHUNK * C], F16)
        nc.sync.dma_start(wkt_s.rearrange("p (a b) -> p a b", a=NCHUNK),
                          wkt.transpose([1, 0, 2]))
        wvt_s = consts.tile([128, NCHUNK * C], F16)
        nc.sync.dma_start(wvt_s.rearrange("p (a b) -> p a b", a=NCHUNK),
                          wvt.transpose([1, 0, 2]))
        bq_s = consts.tile([128, NCHUNK], F32)
        nc.sync.dma_start(bq_s[:], bq2[:])
        bk_s = consts.tile([128, NCHUNK], F32)
        nc.sync.dma_start(bk_s[:], bk2[:])
        bvr_s = consts.tile([1, C], F16)
        nc.sync.dma_start(bvr_s[:], bvr[:])
        u2a_s = consts.tile([T, HW], F16)
        nc.sync.dma_start(u2a_s[:], u2a[:])
        id64_s = consts.tile([T, T], F16)
        nc.sync.dma_start(id64_s[:], id64[:])
        ones_s = consts.tile([1, T], F16)
        nc.sync.dma_start(ones_s[:], ones64[:])

        for b in range(BPC):
            # ---- load (1-a)*F_rgb and F_d (fp16, pre-cast on host) ----
            fr_t = fr_pool.tile([128, NCHUNK * HW], F16, tag="fr")
            nc.sync.dma_start(fr_t.rearrange("p (a b) -> p a b", a=NCHUNK),
                              frgb[b].transpose([1, 0, 2]))
            fd_t = fd_pool.tile([128, NCHUNK * HW], F16, tag="fd")
            nc.sync.dma_start(fd_t.rearrange("p (a b) -> p a b", a=NCHUNK),
                              fd[b].transpose([1, 0, 2]))

            # ---- avgpool sum trees (fp16) ----
            rs_t = small.tile([128, NCHUNK * T], F16, tag="rs")
            ds_t = small.tile([128, NCHUNK * T], F16, tag="ds")
            for src_t, dst_t in ((fr_t, rs_t), (fd_t, ds_t)):
                for ci in range(NCHUNK):
                    _pool_tree(nc, scr_pool,
                               src_t[:, ci * HW:(ci + 1) * HW],
                               dst_t[:, ci * T:(ci + 1) * T])

            # ---- Q, K: [o, s] with per-partition bias (fp16 out) ----
            qf_t = small.tile([128, NCHUNK * T], F16, tag="qf")
            kf_t = small.tile([128, NCHUNK * T], F16, tag="kf")
            for w_s, b_s, sums, dst in ((wqt_s, bq_s, rs_t, qf_t),
                                        (wkt_s, bk_s, ds_t, kf_t)):
                for oj in range(NCHUNK):
                    psq = ps_small.tile([128, T], F32, tag="pss")
                    for ci in range(NCHUNK):
                        nc.tensor.matmul(
                            psq[:],
                            w_s[:, ci * C + oj * 128: ci * C + (oj + 1) * 128],
                            sums[:, ci * T:(ci + 1) * T],
                            start=(ci == 0), stop=(ci == NCHUNK - 1))
                    nc.scalar.activation(
                        dst[:, oj * T:(oj + 1) * T], psq[:],
                        mybir.ActivationFunctionType.Identity,
                        bias=b_s[:, oj:oj + 1], scale=1.0)

            # ---- VfT = D^T Wv^T + ones^T bv : [s, o] ----
            psv = ps_small.tile([T, C], F32, tag="pss")
            for ci in range(NCHUNK):
                nc.tensor.matmul(psv[:],
                                 ds_t[:, ci * T:(ci + 1) * T],
                                 wvt_s[:, ci * C:(ci + 1) * C],
                                 start=(ci == 0), stop=False)
            nc.tensor.matmul(psv[:], ones_s[:], bvr_s[:], start=False, stop=True)
            vft = small.tile([T, C], F16, tag="vft")
            nc.scalar.copy(vft[:], psv[:])

            # ---- A = Qf^T Kf : [t, s] ----
            psa = ps_small.tile([T, T], F32, tag="pss")
            for oj in range(NCHUNK):
                nc.tensor.matmul(psa[:],
                                 qf_t[:, oj * T:(oj + 1) * T],
                                 kf_t[:, oj * T:(oj + 1) * T],
                                 start=(oj == 0), stop=(oj == NCHUNK - 1))

            # ---- softmax over free dim ----
            negmax = small.tile([T, 1], F32, tag="negmax")
            nc.vector.tensor_reduce(negmax[:], psa[:],
                                    axis=mybir.AxisListType.X,
                                    op=mybir.AluOpType.max, negate=True)
            e_t = small.tile([T, T], F32, tag="e")
            nc.scalar.activation(e_t[:], psa[:],
                                 mybir.ActivationFunctionType.Exp,
                                 bias=negmax[:, 0:1], scale=1.0)
            s1 = small.tile([T, 1], F32, tag="s1")
            nc.vector.reduce_sum(s1[:], e_t[:], axis=mybir.AxisListType.X)
            r1 = small.tile([T, 1], F32, tag="r1")
            nc.vector.reciprocal(r1[:], s1[:])
            asm = small.tile([T, T], F16, tag="asm")
            nc.scalar.mul(asm[:], e_t[:], r1[:, 0:1])

            # ---- AsmT via PE transpose ----
            psat = ps_small.tile([T, T], F16, tag="psst", bufs=1)
            nc.tensor.transpose(psat[:], asm[:], id64_s[:])
            asmt = small.tile([T, T], F16, tag="asmt")
            nc.scalar.copy(asmt[:], psat[:])

            # ---- FattT = AsmT @ VfT : [t, c] ----
            psf = ps_small.tile([T, C], F32, tag="pss")
            nc.tensor.matmul(psf[:], asmt[:], vft[:], start=True, stop=True)
            ft = small.tile([T, C], F16, tag="ft")
            nc.scalar.copy(ft[:], psf[:])

            # ---- upsample (PE) + blend add (DVE) -> fp16 out, store ----
            out_t = out_pool.tile([128, NCHUNK * HW], F16, tag="ot")
            for ci in range(NCHUNK):
                for nb in range(HW // 512):
                    off = ci * HW + nb * 512
                    pso = ps_out.tile([128, 512], F32, tag="pso")
                    nc.tensor.matmul(
                        pso[:],
                        ft[:, ci * 128:(ci + 1) * 128],
                        u2a_s[:, nb * 512:(nb + 1) * 512],
                        start=True, stop=True)
                    nc.scalar.copy(out_t[:, off:off + 512], pso[:])
                    if blend:
                        nc.vector.tensor_add(out_t[:, off:off + 512],
                                             out_t[:, off:off + 512],
                                             fr_t[:, off:off + 512])
            nc.sync.dma_start(out[b].transpose([1, 0, 2]),
                              out_t.rearrange("p (a b) -> p a b", a=NCHUNK))

    nc.compile()
    return nc


def _prepare_in_maps(F_rgb, F_d, Wq, bq, Wk, bk, Wv, bv, alpha):
    if "U" not in _CACHE:
        _CACHE["U"] = _bilinear_up_matrix(H, AS)
    U = _CACHE["U"]

    a = float(np.asarray(alpha))
    blend = abs(1.0 - a) > 1e-7
    rscale = (1.0 - a) if blend else 1.0

    F_rgb = (np.asarray(F_rgb, np.float32) * np.float32(rscale)).astype(NPF16)
    F_d = np.asarray(F_d, np.float32).astype(NPF16)

    frgb_sh = F_rgb.reshape(NCORES, BPC, NCHUNK, 128, HW)
    fd_sh = F_d.reshape(NCORES, BPC, NCHUNK, 128, HW)

    def wfold(Wx, extra=1.0):
        # [c, o] chunks of (Wx / 64 / extra)^T
        return np.ascontiguousarray(
            (np.asarray(Wx, np.float64).T / (AS * AS * extra))
            .reshape(NCHUNK, 128, C)).astype(NPF16)

    wqt = wfold(Wq, extra=rscale)   # R sums are pre-scaled by rscale
    wkt = wfold(Wk)
    wvt = wfold(Wv)
    bq2 = np.ascontiguousarray(np.asarray(bq, np.float32).reshape(NCHUNK, 128).T)
    bk2 = np.ascontiguousarray(np.asarray(bk, np.float32).reshape(NCHUNK, 128).T)
    bvr = np.asarray(bv, np.float32).reshape(1, C).astype(NPF16)
    u2a = (a * np.kron(U.T, U.T)).astype(NPF16)
    id64 = np.eye(T, dtype=np.float32).astype(NPF16)
    ones64 = np.ones((1, T), NPF16)

    in_maps = []
    for i in range(NCORES):
        in_maps.append({
            "frgb": np.ascontiguousarray(frgb_sh[i]),
            "fd": np.ascontiguousarray(fd_sh[i]),
            "wqt": wqt, "wkt": wkt, "wvt": wvt,
            "bq2": bq2, "bk2": bk2, "bvr": bvr,
            "u2a": u2a, "id64": id64, "ones64": ones64,
        })
    return in_maps, blend


def _execute(in_maps, blend=True, **kwargs):
    key = f"nc_{blend}"
    if key not in _CACHE:
        _CACHE[key] = _build_program(blend)
    res = run_bass_kernel_spmd(_CACHE[key], in_maps, list(range(NCORES)),
                               **kwargs)
    parts = [res.results[i]["out"].astype(np.float32).reshape(BPC, C, H, W)
             for i in range(NCORES)]
    return np.concatenate(parts, axis=0), res


def kernel(F_rgb, F_d, Wq, bq, Wk, bk, Wv, bv, alpha):
    in_maps, blend = _prepare_in_maps(F_rgb, F_d, Wq, bq, Wk, bk, Wv, bv,
                                      alpha)
    out, _ = _execute(in_maps, blend=blend)
    return out
